# revision 4
# baseline (speedup 1.0000x reference)
"""AdaPT int8-quantized Linear on 8 TRN2 NeuronCores.

out = round_int8(x * 127/amax(x)) @ round_int8(w * 127/amax(w)).T * dequant + bias

Exactness: int8 values (|v| <= 127) are exact in bf16; their products
(<= 16129) and the accumulated partial sums are exact in fp32 PSUM, so a
bf16 TensorE matmul reproduces the int8 x int8 -> int32 matmul bit-exactly
at full bf16 throughput. round() is implemented as (v*scale + 1.5*2^23) -
1.5*2^23 in f32 (round-half-even, matching jnp). Rounding-sensitive math
runs on VectorE/GpSimdE only (both IEEE f32; ScalarE's activation affine
pre-op is not exact); abs/max are rounding-free.

Pipeline layout (v2, prologue-optimized):
- ALL input loads on one DMA queue (sync) in priority order:
  w-amax slice (8MB) -> x-amax slice (16MB) -> w panel 0 (8MB) ->
  x re-read nb-major (16MB) -> w panels 1..7.  Output writes on gpsimd.
- Two tiny AllReduce-max collectives; AR_w launches ~35us in (absorbs
  cross-core skew), AR_x takes the fast path right after the x scan.
  A dummy "primer" AllReduce at t=0 absorbs CC-engine setup + launch skew.
- x is re-read in n-block-major [4096k x 128n] chunks so quantized x
  becomes available per n-block and the first matmul can start as soon as
  scale_x + panel-0 chunk 0 are ready (~105us vs 203us in v1).
- Quantize passes split across VectorE (pass 1) and GpSimdE (pass 2),
  alternating per chunk, so neither engine gates the pipeline.
- Tiny paced dummy matmuls (f32, N=1) keep the PE HAM warm through the
  prologue so the first real matmuls issue at full clock.

x row-parallel: core c computes out rows [c*1024, (c+1)*1024).
"""

import numpy as np

import concourse.bass as bass
import concourse.bacc as bacc
import concourse.bass_isa as bass_isa
import concourse.mybir as mybir
import concourse.tile as tile
from concourse.bass_utils import run_bass_kernel_spmd

N, K, M = 8192, 4096, 4096
N_CORES = 8
NS = N // N_CORES   # 1024 x rows per core
WS = M // N_CORES   # 512 w rows per core (amax shard)
P = 128
KB = K // P         # 32 k-blocks
NB = NS // P        # 8 n-blocks per core
MP = 512            # m-panel width
NMP = M // MP       # 8 m-panels

MAGIC = 12582912.0  # 1.5 * 2**23
F32 = mybir.dt.float32
BF16 = mybir.dt.bfloat16

_cached_nc = None


def _body(nc, tc, xs, wa, wf, bias_in, out):
    RG = [list(range(N_CORES))]
    # xs: [K, NS] f32 (x.T slice): amax tiles [16][128, 2, 1024]
    # wa: [K, WS] f32 (w.T slice): amax tiles [8][128, 4, 512]
    xa_t = xs.rearrange("(t a p) n -> t p a n", a=2, p=P)   # [16, 128, 2, 1024]
    wa_t = wa.rearrange("(h a p) m -> h p a m", a=4, p=P)   # [8, 128, 4, 512]

    with (
        tc.tile_pool(name="const", bufs=1) as const,
        tc.tile_pool(name="dram", bufs=1, space="DRAM") as dram,
        tc.tile_pool(name="ld", bufs=6) as ld,
        tc.tile_pool(name="xt", bufs=1) as xtp,
        tc.tile_pool(name="wt", bufs=8) as wtp,
        tc.tile_pool(name="ps", bufs=7, space="PSUM") as psp,
        tc.tile_pool(name="psd", bufs=1, space="PSUM") as psdp,
        tc.tile_pool(name="ob", bufs=4) as obp,
    ):
        ccp_in = dram.tile([1, 16], F32)
        ccp_out = dram.tile([1, 16], F32, addr_space="Shared")
        ccw_in = dram.tile([1, 16], F32)
        ccw_out = dram.tile([1, 16], F32, addr_space="Shared")
        ccx_in = dram.tile([1, 16], F32)
        ccx_out = dram.tile([1, 16], F32, addr_space="Shared")
        bias_bc = const.tile([P, M], F32)
        scl = const.tile([P, 4], F32)   # 0:scale_x 1:scale_w 2:dequant 3:tmp
        psd = psdp.tile([P, 16], F32)   # dummy-matmul scratch bank

        def dummy_mm(dep_ap):
            # Tiny f32 matmul reading dep_ap: paced by the producer of
            # dep_ap, keeps the PE HAM activity window warm pre-main-loop.
            nc.tensor.matmul(psd[0:1, 0:1], dep_ap, dep_ap,
                             start=True, stop=True)

        # ---- primer AllReduce: absorbs CC setup + launch skew ----
        primer = const.tile([1, 16], F32)
        nc.vector.memset(primer[:], 0.0)
        nc.gpsimd.dma_start(ccp_in[:], primer[:])
        nc.gpsimd.collective_compute(
            "AllReduce", mybir.AluOpType.max,
            ins=[ccp_in.opt()], outs=[ccp_out.opt()], replica_groups=RG,
        )

        bias_b_ap = bass.AP(
            tensor=bias_in.tensor,
            offset=bias_in.offset,
            ap=[[0, P]] + list(bias_in.ap),
        )
        nc.gpsimd.dma_start(out=bias_bc[:], in_=bias_b_ap)

        # ---- w abs-max first (8MB): its AllReduce absorbs the skew ----
        partw = const.tile([P, 8], F32)
        partx = const.tile([P, 16], F32)
        for h in range(8):
            tl = ld.tile([P, 4, WS], F32, tag="ld", name=f"ldw{h}")
            nc.sync.dma_start(tl[:], wa_t[h])
            nc.vector.tensor_reduce(
                out=partw[:, h : h + 1], in_=tl[:], op=mybir.AluOpType.max,
                axis=mybir.AxisListType.XY, apply_absolute_value=True,
            )
            dummy_mm(partw[:, h : h + 1])
        pw = const.tile([P, 1], F32)
        nc.vector.tensor_reduce(out=pw[:], in_=partw[:], op=mybir.AluOpType.max,
                                axis=mybir.AxisListType.X)
        rw = const.tile([P, 1], F32)
        nc.gpsimd.partition_all_reduce(rw[:], pw[:], channels=P,
                                       reduce_op=bass_isa.ReduceOp.max)
        packw = const.tile([1, 16], F32)
        nc.vector.memset(packw[:], 0.0)
        nc.vector.tensor_copy(packw[:1, 0:1], rw[:1, :])
        nc.gpsimd.dma_start(ccw_in[:], packw[:])
        nc.gpsimd.collective_compute(
            "AllReduce", mybir.AluOpType.max,
            ins=[ccw_in.opt()], outs=[ccw_out.opt()], replica_groups=RG,
        )

        # ---- x abs-max (16MB) ----
        for t in range(16):
            tl = ld.tile([P, 2, NS], F32, tag="ld", name=f"ldx{t}")
            nc.sync.dma_start(tl[:], xa_t[t])
            nc.vector.tensor_reduce(
                out=partx[:, t : t + 1], in_=tl[:], op=mybir.AluOpType.max,
                axis=mybir.AxisListType.XY, apply_absolute_value=True,
            )
            dummy_mm(partx[:, t : t + 1])
        px = const.tile([P, 1], F32)
        nc.vector.tensor_reduce(out=px[:], in_=partx[:], op=mybir.AluOpType.max,
                                axis=mybir.AxisListType.X)
        rx = const.tile([P, 1], F32)
        nc.gpsimd.partition_all_reduce(rx[:], px[:], channels=P,
                                       reduce_op=bass_isa.ReduceOp.max)
        packx = const.tile([1, 16], F32)
        nc.vector.memset(packx[:], 0.0)
        nc.vector.tensor_copy(packx[:1, 0:1], rx[:1, :])
        nc.gpsimd.dma_start(ccx_in[:], packx[:])
        nc.gpsimd.collective_compute(
            "AllReduce", mybir.AluOpType.max,
            ins=[ccx_in.opt()], outs=[ccx_out.opt()], replica_groups=RG,
        )

        # ---- scale_w (ready ~AR_w completion, well before scale_x) ----
        gotw = const.tile([1, 16], F32)
        nc.gpsimd.dma_start(gotw[:], ccw_out[:])
        gbw = const.tile([P, 16], F32)
        nc.gpsimd.partition_broadcast(gbw[:], gotw[:])
        invw = const.tile([P, 1], F32)
        nc.vector.reciprocal(invw[:], gbw[:, 0:1])
        nc.vector.tensor_scalar(out=scl[:, 1:2], in0=invw[:], scalar1=127.0,
                                scalar2=None, op0=mybir.AluOpType.mult)
        dummy_mm(scl[:, 1:2])

        # ---- panel-0 w: load + quantize (gated only on scale_w) ----
        def load_panel(p):
            tls = []
            for g in range(8):
                tl = ld.tile([P, 4, MP], F32, tag="ld", name=f"wld{p}_{g}")
                src = bass.AP(
                    tensor=wf.tensor,
                    offset=wf.offset + g * (K // 8) * M + p * MP,
                    ap=[[M, P], [P * M, 4], [1, MP]],
                )
                nc.sync.dma_start(tl[:], src)
                tls.append(tl)
            return tls

        def quant_chunk(p, g, tl, wq, warm=False):
            e1 = nc.vector if g % 2 == 0 else nc.gpsimd
            e2 = nc.gpsimd if g % 2 == 0 else nc.vector
            e1.tensor_scalar(out=tl[:], in0=tl[:], scalar1=scl[:, 1:2],
                             scalar2=MAGIC, op0=mybir.AluOpType.mult,
                             op1=mybir.AluOpType.add)
            dst = wq[g // 2][:, 4 * (g % 2) : 4 * (g % 2) + 4, :]
            e2.tensor_scalar(out=dst, in0=tl[:], scalar1=MAGIC,
                             scalar2=None, op0=mybir.AluOpType.subtract)
            if warm:
                dummy_mm(wq[g // 2][:, 4 * (g % 2), 0:1])

        wq_cur = [wtp.tile([P, 8, MP], BF16, tag="wq", name=f"wq0_{h}")
                  for h in range(4)]
        tls0 = load_panel(0)
        for g in range(8):
            quant_chunk(0, g, tls0[g], wq_cur, warm=True)

        # ---- scale_x + dequant ----
        gotx = const.tile([1, 16], F32)
        nc.gpsimd.dma_start(gotx[:], ccx_out[:])
        gbx = const.tile([P, 16], F32)
        nc.gpsimd.partition_broadcast(gbx[:], gotx[:])
        invx = const.tile([P, 1], F32)
        nc.vector.reciprocal(invx[:], gbx[:, 0:1])
        nc.vector.tensor_scalar(out=scl[:, 0:1], in0=invx[:], scalar1=127.0,
                                scalar2=None, op0=mybir.AluOpType.mult)
        nc.vector.tensor_tensor(out=scl[:, 3:4], in0=gbx[:, 0:1], in1=gbw[:, 0:1],
                                op=mybir.AluOpType.mult)
        nc.vector.tensor_scalar(out=scl[:, 2:3], in0=scl[:, 3:4],
                                scalar1=float(np.float32(1.0) / np.float32(16129.0)),
                                scalar2=None, op0=mybir.AluOpType.mult)

        # ---- x re-read, n-block-major; quantize into resident xT ----
        xT = xtp.tile([P, KB, NS], BF16)  # resident quantized x.T (8.4 MB)
        xre_tiles = []
        for nb in range(NB):
            halves = []
            for a in range(2):
                tl = ld.tile([P, 16, P], F32, tag="ld", name=f"xre{nb}_{a}")
                src = bass.AP(
                    tensor=xs.tensor,
                    offset=xs.offset + a * 16 * P * NS + nb * P,
                    ap=[[NS, P], [P * NS, 16], [1, P]],
                )
                nc.sync.dma_start(tl[:], src)
                halves.append(tl)
            xre_tiles.append(halves)

        def quant_x(nb):
            for a in range(2):
                tl = xre_tiles[nb][a]
                e1 = nc.vector if a == 0 else nc.gpsimd
                e2 = nc.gpsimd if a == 0 else nc.vector
                e1.tensor_scalar(out=tl[:], in0=tl[:], scalar1=scl[:, 0:1],
                                 scalar2=MAGIC, op0=mybir.AluOpType.mult,
                                 op1=mybir.AluOpType.add)
                e2.tensor_scalar(
                    out=xT[:, a * 16 : a * 16 + 16, nb * P : (nb + 1) * P],
                    in0=tl[:], scalar1=MAGIC, scalar2=None,
                    op0=mybir.AluOpType.subtract)

        quant_x(0)
        quant_x(1)

        # ---- main loop: 8 panels x 8 n-blocks x 32 k-steps ----
        for p in range(NMP):
            if p + 1 < NMP:
                tls_next = load_panel(p + 1)
                wq_next = [wtp.tile([P, 8, MP], BF16, tag="wq",
                                    name=f"wq{p + 1}_{h}") for h in range(4)]
            for nb in range(NB):
                if p == 0 and nb + 2 < NB:
                    quant_x(nb + 2)
                ps = psp.tile([P, MP], F32, tag="ps", name=f"ps{p}_{nb}")
                for i in range(KB):
                    ks = (4 * nb + i) % KB
                    nc.tensor.matmul(
                        ps[:], xT[:, ks, nb * P : (nb + 1) * P],
                        wq_cur[ks // 8][:, ks % 8, :],
                        start=(i == 0), stop=(i == KB - 1),
                    )
                ob = obp.tile([P, MP], F32, tag="ob", name=f"ob{p}_{nb}")
                nc.vector.scalar_tensor_tensor(
                    out=ob[:], in0=ps[:], scalar=scl[:, 2:3],
                    in1=bias_bc[:, p * MP : (p + 1) * MP],
                    op0=mybir.AluOpType.mult, op1=mybir.AluOpType.add,
                )
                nc.gpsimd.dma_start(
                    out[nb * P : (nb + 1) * P, p * MP : (p + 1) * MP], ob[:]
                )
                # pipeline next panel's quantize; panel 0 keeps V/G free for
                # x-quant early, so push its w-quant to the window tail
                if p + 1 < NMP:
                    if p == 0:
                        sched = {5: (0, 1), 6: (2, 3, 4), 7: (5, 6, 7)}
                    else:
                        sched = {2: (0, 1), 3: (2, 3), 4: (4, 5), 5: (6, 7)}
                    for g in sched.get(nb, ()):
                        quant_chunk(p + 1, g, tls_next[g], wq_next)
            if p + 1 < NMP:
                wq_cur = wq_next


def _build():
    global _cached_nc
    if _cached_nc is not None:
        return _cached_nc
    nc = bacc.Bacc("TRN2", target_bir_lowering=False, debug=False,
                   num_devices=N_CORES)
    xs = nc.dram_tensor("xs", [K, NS], F32, kind="ExternalInput")
    wa = nc.dram_tensor("wa", [K, WS], F32, kind="ExternalInput")
    wf = nc.dram_tensor("wf", [K, M], F32, kind="ExternalInput")
    bias = nc.dram_tensor("bias", [M], F32, kind="ExternalInput")
    out = nc.dram_tensor("out", [NS, M], F32, kind="ExternalOutput")
    with tile.TileContext(nc) as tc:
        _body(nc, tc, xs.ap(), wa.ap(), wf.ap(), bias.ap(), out.ap())
    nc.compile()
    _cached_nc = nc
    return nc


def kernel(x, weight, bias, _trace=False, _trace_kwargs=None):
    x = np.asarray(x, dtype=np.float32)
    weight = np.asarray(weight, dtype=np.float32)
    bias = np.ascontiguousarray(np.asarray(bias, dtype=np.float32))
    assert x.shape == (N, K) and weight.shape == (M, K) and bias.shape == (M,)

    nc = _build()
    xt = x.T                              # [K, N] view
    wt = np.ascontiguousarray(weight.T)   # [K, M]
    in_maps = [
        {
            "xs": np.ascontiguousarray(xt[:, c * NS : (c + 1) * NS]),
            "wa": np.ascontiguousarray(wt[:, c * WS : (c + 1) * WS]),
            "wf": wt,
            "bias": bias,
        }
        for c in range(N_CORES)
    ]
    res = run_bass_kernel_spmd(
        nc, in_maps, core_ids=list(range(N_CORES)),
        trace=_trace, **(_trace_kwargs or {}),
    )
    out = np.concatenate([res.results[c]["out"] for c in range(N_CORES)], axis=0)
    if _trace:
        return out, res
    return out


# revision 5
# speedup vs baseline: 2.4928x; 2.4928x over previous
"""AdaPT int8-quantized Linear on 8 TRN2 NeuronCores.

out = round_int8(x * 127/amax(x)) @ round_int8(w * 127/amax(w)).T * dequant + bias

Exactness: int8 values (|v| <= 127) are exact in bf16; their products
(<= 16129) and the accumulated partial sums are exact in fp32 PSUM, so a
bf16 TensorE matmul reproduces the int8 x int8 -> int32 matmul bit-exactly
at full bf16 throughput. round() is implemented as (v*scale + 1.5*2^23) -
1.5*2^23 in f32 (round-half-even, matching jnp) on VectorE only (ScalarE's
activation affine pre-op is not exact; GpSimd tensor ops contend with DVE
for the shared SBUF port pair and fully block -- measured 25x slowdown).

Pipeline layout (v3, prologue-optimized):
- ALL input loads ride one HWDGE queue (sync) in priority order:
  w-amax slice (8MB) -> x-amax slice (16MB) -> w panel 0 (8MB) ->
  x re-read (16MB) -> w panels 1..7.  Output writes ride the other HWDGE
  queue (scalar/ACT), so neither competes with the loads nor with GpSimd.
- Two tiny AllReduce-max collectives, w first: AR_w's mesh absorbs the
  CC-engine warmup + cross-core skew while the x scan still streams;
  AR_x then takes the short path, so scale_x lands ~115us.
- x is re-read in n-block-major [4096k x 128n] chunks; quantized x lands
  per n-block in a block-contiguous xT [P][nb][ks][128] so every Vector
  op is a single contiguous run (strided sub-row writes measured 16x
  slower). First matmul starts ~127us (vs 203us baseline).
- Tiny paced dummy matmuls keep the PE HAM warm through the prologue.

x row-parallel: core c computes out rows [c*1024, (c+1)*1024).
"""

import numpy as np

import concourse.bass as bass
import concourse.bacc as bacc
import concourse.bass_isa as bass_isa
import concourse.mybir as mybir
import concourse.tile as tile
from concourse.bass_utils import run_bass_kernel_spmd

N, K, M = 8192, 4096, 4096
N_CORES = 8
NS = N // N_CORES   # 1024 x rows per core
WS = M // N_CORES   # 512 w rows per core (amax shard)
P = 128
KB = K // P         # 32 k-blocks
NB = NS // P        # 8 n-blocks per core
MP = 512            # m-panel width
NMP = M // MP       # 8 m-panels

MAGIC = 12582912.0  # 1.5 * 2**23
F32 = mybir.dt.float32
BF16 = mybir.dt.bfloat16

_cached_nc = None


def _body(nc, tc, xs, wa, wf, bias_in, out):
    RG = [list(range(N_CORES))]
    xa_t = xs.rearrange("(t a p) n -> t p a n", a=2, p=P)   # [16, 128, 2, 1024]
    wa_t = wa.rearrange("(h a p) m -> h p a m", a=4, p=P)   # [8, 128, 4, 512]

    with (
        tc.tile_pool(name="const", bufs=1) as const,
        tc.tile_pool(name="dram", bufs=1, space="DRAM") as dram,
        tc.tile_pool(name="ld", bufs=6) as ld,
        tc.tile_pool(name="xt", bufs=1) as xtp,
        tc.tile_pool(name="wt", bufs=8) as wtp,
        tc.tile_pool(name="ps", bufs=7, space="PSUM") as psp,
        tc.tile_pool(name="psd", bufs=1, space="PSUM") as psdp,
        tc.tile_pool(name="ob", bufs=4) as obp,
    ):
        ccw_in = dram.tile([1, 16], F32)
        ccw_out = dram.tile([1, 16], F32, addr_space="Shared")
        ccx_in = dram.tile([1, 16], F32)
        ccx_out = dram.tile([1, 16], F32, addr_space="Shared")
        bias_bc = const.tile([P, M], F32)
        scl = const.tile([P, 4], F32)   # 0:scale_x 1:scale_w 2:dequant 3:tmp
        psd = psdp.tile([P, 16], F32)   # dummy-matmul scratch bank

        def dummy_mm(dep_ap):
            # Tiny matmul reading dep_ap: paced by dep_ap's producer, keeps
            # the PE HAM activity window warm before the main loop.
            nc.tensor.matmul(psd[0:1, 0:1], dep_ap, dep_ap,
                             start=True, stop=True)

        bias_b_ap = bass.AP(
            tensor=bias_in.tensor,
            offset=bias_in.offset,
            ap=[[0, P]] + list(bias_in.ap),
        )
        nc.gpsimd.dma_start(out=bias_bc[:], in_=bias_b_ap)

        # ---- w abs-max first (8MB): its AllReduce absorbs CC warmup ----
        partw = const.tile([P, 8], F32)
        partx = const.tile([P, 16], F32)
        for h in range(8):
            tl = ld.tile([P, 4, WS], F32, tag="ld", name=f"ldw{h}")
            nc.sync.dma_start(tl[:], wa_t[h])
            nc.vector.tensor_reduce(
                out=partw[:, h : h + 1], in_=tl[:], op=mybir.AluOpType.max,
                axis=mybir.AxisListType.XY, apply_absolute_value=True,
            )
            dummy_mm(partw[:, h : h + 1])
        pw = const.tile([P, 1], F32)
        nc.vector.tensor_reduce(out=pw[:], in_=partw[:], op=mybir.AluOpType.max,
                                axis=mybir.AxisListType.X)
        rw = const.tile([P, 1], F32)
        nc.gpsimd.partition_all_reduce(rw[:], pw[:], channels=P,
                                       reduce_op=bass_isa.ReduceOp.max)
        packw = const.tile([1, 16], F32)
        nc.vector.memset(packw[:], 0.0)
        nc.vector.tensor_copy(packw[:1, 0:1], rw[:1, :])
        nc.gpsimd.dma_start(ccw_in[:], packw[:])
        nc.gpsimd.collective_compute(
            "AllReduce", mybir.AluOpType.max,
            ins=[ccw_in.opt()], outs=[ccw_out.opt()], replica_groups=RG,
        )

        # ---- x abs-max (16MB) ----
        for t in range(16):
            tl = ld.tile([P, 2, NS], F32, tag="ld", name=f"ldx{t}")
            nc.sync.dma_start(tl[:], xa_t[t])
            nc.vector.tensor_reduce(
                out=partx[:, t : t + 1], in_=tl[:], op=mybir.AluOpType.max,
                axis=mybir.AxisListType.XY, apply_absolute_value=True,
            )
            dummy_mm(partx[:, t : t + 1])
        px = const.tile([P, 1], F32)
        nc.vector.tensor_reduce(out=px[:], in_=partx[:], op=mybir.AluOpType.max,
                                axis=mybir.AxisListType.X)
        rx = const.tile([P, 1], F32)
        nc.gpsimd.partition_all_reduce(rx[:], px[:], channels=P,
                                       reduce_op=bass_isa.ReduceOp.max)
        packx = const.tile([1, 16], F32)
        nc.vector.memset(packx[:], 0.0)
        nc.vector.tensor_copy(packx[:1, 0:1], rx[:1, :])
        nc.gpsimd.dma_start(ccx_in[:], packx[:])
        nc.gpsimd.collective_compute(
            "AllReduce", mybir.AluOpType.max,
            ins=[ccx_in.opt()], outs=[ccx_out.opt()], replica_groups=RG,
        )

        # ---- scale_w (ready at AR_w completion, before scale_x) ----
        gotw = const.tile([1, 16], F32)
        nc.gpsimd.dma_start(gotw[:], ccw_out[:])
        gbw = const.tile([P, 16], F32)
        nc.gpsimd.partition_broadcast(gbw[:], gotw[:])
        invw = const.tile([P, 1], F32)
        nc.vector.reciprocal(invw[:], gbw[:, 0:1])
        nc.vector.tensor_scalar(out=scl[:, 1:2], in0=invw[:], scalar1=127.0,
                                scalar2=None, op0=mybir.AluOpType.mult)
        dummy_mm(scl[:, 1:2])

        # ---- panel-0 w: load + quantize (gated only on scale_w) ----
        def load_panel(p):
            tls = []
            for g in range(8):
                tl = ld.tile([P, 4, MP], F32, tag="ld", name=f"wld{p}_{g}")
                src = bass.AP(
                    tensor=wf.tensor,
                    offset=wf.offset + g * (K // 8) * M + p * MP,
                    ap=[[M, P], [P * M, 4], [1, MP]],
                )
                nc.sync.dma_start(tl[:], src)
                tls.append(tl)
            return tls

        def quant_chunk(p, g, tl, wq, warm=False):
            nc.vector.tensor_scalar(out=tl[:], in0=tl[:], scalar1=scl[:, 1:2],
                                    scalar2=MAGIC, op0=mybir.AluOpType.mult,
                                    op1=mybir.AluOpType.add)
            dst = wq[g // 2][:, 4 * (g % 2) : 4 * (g % 2) + 4, :]
            nc.vector.tensor_scalar(out=dst, in0=tl[:], scalar1=MAGIC,
                                    scalar2=None, op0=mybir.AluOpType.subtract)
            if warm:
                dummy_mm(wq[g // 2][:, 4 * (g % 2), 0:1])

        wq_cur = [wtp.tile([P, 8, MP], BF16, tag="wq", name=f"wq0_{h}")
                  for h in range(4)]
        tls0 = load_panel(0)
        for g in range(8):
            quant_chunk(0, g, tls0[g], wq_cur, warm=True)

        # ---- scale_x + dequant ----
        gotx = const.tile([1, 16], F32)
        nc.gpsimd.dma_start(gotx[:], ccx_out[:])
        gbx = const.tile([P, 16], F32)
        nc.gpsimd.partition_broadcast(gbx[:], gotx[:])
        invx = const.tile([P, 1], F32)
        nc.vector.reciprocal(invx[:], gbx[:, 0:1])
        nc.vector.tensor_scalar(out=scl[:, 0:1], in0=invx[:], scalar1=127.0,
                                scalar2=None, op0=mybir.AluOpType.mult)
        nc.vector.tensor_tensor(out=scl[:, 3:4], in0=gbx[:, 0:1], in1=gbw[:, 0:1],
                                op=mybir.AluOpType.mult)
        nc.vector.tensor_scalar(out=scl[:, 2:3], in0=scl[:, 3:4],
                                scalar1=float(np.float32(1.0) / np.float32(16129.0)),
                                scalar2=None, op0=mybir.AluOpType.mult)

        # ---- x re-read, n-block-major; quantize into blocked xT ----
        # xT layout [P][nb][ks][128]: per-nb writes are one contiguous run.
        xT = xtp.tile([P, NB, KB, P], BF16)  # resident quantized x.T (8.4 MB)
        xre_tiles = []
        for nb in range(NB):
            halves = []
            for a in range(2):
                tl = ld.tile([P, 16, P], F32, tag="ld", name=f"xre{nb}_{a}")
                src = bass.AP(
                    tensor=xs.tensor,
                    offset=xs.offset + a * 16 * P * NS + nb * P,
                    ap=[[NS, P], [P * NS, 16], [1, P]],
                )
                nc.sync.dma_start(tl[:], src)
                halves.append(tl)
            xre_tiles.append(halves)

        def quant_x(nb):
            for a in range(2):
                tl = xre_tiles[nb][a]
                nc.vector.tensor_scalar(out=tl[:], in0=tl[:], scalar1=scl[:, 0:1],
                                        scalar2=MAGIC, op0=mybir.AluOpType.mult,
                                        op1=mybir.AluOpType.add)
                nc.vector.tensor_scalar(
                    out=xT[:, nb, a * 16 : a * 16 + 16, :],
                    in0=tl[:], scalar1=MAGIC, scalar2=None,
                    op0=mybir.AluOpType.subtract)

        quant_x(0)
        quant_x(1)

        # ---- main loop: 8 panels x 8 n-blocks x 32 k-steps ----
        for p in range(NMP):
            if p + 1 < NMP:
                tls_next = load_panel(p + 1)
                wq_next = [wtp.tile([P, 8, MP], BF16, tag="wq",
                                    name=f"wq{p + 1}_{h}") for h in range(4)]
            for nb in range(NB):
                if p == 0 and nb + 2 < NB:
                    quant_x(nb + 2)
                ps = psp.tile([P, MP], F32, tag="ps", name=f"ps{p}_{nb}")
                for i in range(KB):
                    ks = (4 * nb + i) % KB
                    nc.tensor.matmul(
                        ps[:], xT[:, nb, ks, :],
                        wq_cur[ks // 8][:, ks % 8, :],
                        start=(i == 0), stop=(i == KB - 1),
                    )
                ob = obp.tile([P, MP], F32, tag="ob", name=f"ob{p}_{nb}")
                nc.vector.scalar_tensor_tensor(
                    out=ob[:], in0=ps[:], scalar=scl[:, 2:3],
                    in1=bias_bc[:, p * MP : (p + 1) * MP],
                    op0=mybir.AluOpType.mult, op1=mybir.AluOpType.add,
                )
                nc.scalar.dma_start(
                    out[nb * P : (nb + 1) * P, p * MP : (p + 1) * MP], ob[:]
                )
                # pipeline next panel's quantize; panel 0 keeps Vector free
                # for x-quant early, so push its w-quant to the window tail
                if p + 1 < NMP:
                    if p == 0:
                        sched = {5: (0, 1), 6: (2, 3, 4), 7: (5, 6, 7)}
                    else:
                        sched = {2: (0, 1), 3: (2, 3), 4: (4, 5), 5: (6, 7)}
                    for g in sched.get(nb, ()):
                        quant_chunk(p + 1, g, tls_next[g], wq_next)
            if p + 1 < NMP:
                wq_cur = wq_next


def _build():
    global _cached_nc
    if _cached_nc is not None:
        return _cached_nc
    nc = bacc.Bacc("TRN2", target_bir_lowering=False, debug=False,
                   num_devices=N_CORES)
    xs = nc.dram_tensor("xs", [K, NS], F32, kind="ExternalInput")
    wa = nc.dram_tensor("wa", [K, WS], F32, kind="ExternalInput")
    wf = nc.dram_tensor("wf", [K, M], F32, kind="ExternalInput")
    bias = nc.dram_tensor("bias", [M], F32, kind="ExternalInput")
    out = nc.dram_tensor("out", [NS, M], F32, kind="ExternalOutput")
    with tile.TileContext(nc) as tc:
        _body(nc, tc, xs.ap(), wa.ap(), wf.ap(), bias.ap(), out.ap())
    nc.compile()
    _cached_nc = nc
    return nc


def kernel(x, weight, bias, _trace=False, _trace_kwargs=None):
    x = np.asarray(x, dtype=np.float32)
    weight = np.asarray(weight, dtype=np.float32)
    bias = np.ascontiguousarray(np.asarray(bias, dtype=np.float32))
    assert x.shape == (N, K) and weight.shape == (M, K) and bias.shape == (M,)

    nc = _build()
    xt = x.T                              # [K, N] view
    wt = np.ascontiguousarray(weight.T)   # [K, M]
    in_maps = [
        {
            "xs": np.ascontiguousarray(xt[:, c * NS : (c + 1) * NS]),
            "wa": np.ascontiguousarray(wt[:, c * WS : (c + 1) * WS]),
            "wf": wt,
            "bias": bias,
        }
        for c in range(N_CORES)
    ]
    res = run_bass_kernel_spmd(
        nc, in_maps, core_ids=list(range(N_CORES)),
        trace=_trace, **(_trace_kwargs or {}),
    )
    out = np.concatenate([res.results[c]["out"] for c in range(N_CORES)], axis=0)
    if _trace:
        return out, res
    return out
